# revision 19
# baseline (speedup 1.0000x reference)
"""Quantized matmul (uint4 groupwise dequant) on 8 Trainium2 NeuronCores.

Computes out = a_f32 @ W where W[k, n] = (q[k, n] - zeros[k//128, n]) * scales[k//128, n].

Sharding: tensor-parallel along N (output features). Each of the 8 cores gets
N_LOCAL = 512 columns of q/scales/zeros and the full `a` (replicated). Each
core dequantizes its W slice to fp16 once into SBUF, then runs a dense
fp16 matmul with fp32 PSUM accumulation.

Device kernel layout choices (all host-side prep is pure layout/sharding):
 - `a` is fed pre-transposed and tiled as aT[m_out, k_in, k_out*128 + m_in]
   so each [128, 4096] SBUF tile is one contiguous 1 MiB DMA and slices
   [:, k*128:(k+1)*128] are matmul lhsT tiles (K on partitions).
 - q values are 0..15, so the int32 container is narrowed to int8 on the
   host (lossless) to quarter its DMA cost; the DVE subtract consumes the
   int8 operand directly (int8 - fp16 -> fp16 in one op).
 - scales/zeros come in as [32, 512] slices; both are broadcast across the
   128 partitions on-device with chunked stride-0 DRAM->SBUF DMAs.

Schedule: the PE's first matmuls only need a quarter of m-tile 0, so aT[0]
is loaded as four [128, 1024] sub-tiles. The first 8-m-tile block runs as a
wavefront ordered by estimated operand arrival (aT tiles and W groups
stream in concurrently at ~358 GB/s); later m-tiles run m-outer/k-inner
with inline epilogues so output DMAs spread out instead of bursting at the
tail.
"""

import numpy as np

M, K, N = 4096, 4096, 4096
G = 128          # quant group size
P = 128          # partitions
NCORES = 8
NL = N // NCORES          # 512 output columns per core
KT = K // P               # 32 k tiles (== quant groups)
MT = M // P               # 32 m tiles
MBLK = 8                  # m-tiles in the wavefront block (8 PSUM banks)
AQ = 4                    # aT[0] is split into AQ sub-tiles

_CACHE = {}


def _build_nc():
    import concourse.bacc as bacc
    import concourse.mybir as mybir
    import concourse.tile as tile
    from concourse.bass import ts

    f16 = mybir.dt.float16
    f32 = mybir.dt.float32
    i8 = mybir.dt.int8

    nc = bacc.Bacc("TRN2", target_bir_lowering=False, debug=False)

    aT = nc.dram_tensor("aT", [MT, P, K], f16, kind="ExternalInput").ap()
    q = nc.dram_tensor("q", [KT, P, NL], i8, kind="ExternalInput").ap()
    zsm = nc.dram_tensor("zsm", [1, KT * NL], f16, kind="ExternalInput").ap()
    ssm = nc.dram_tensor("ssm", [1, KT * NL], f16, kind="ExternalInput").ap()
    out = nc.dram_tensor("out", [MT, P, NL], f32, kind="ExternalOutput").ap()

    with tile.TileContext(nc) as tc:
        # Broadcast-chunk sizes (groups per chunk): tiny leading chunks so
        # W_0 is ready right after the DMA engines boot, bigger later ones
        # to keep the dma_start count low.
        CHUNKS = [1, 1, 2, 4, 4, 4, 4, 4, 4, 4]
        assert sum(CHUNKS) == KT
        KQ = KT // AQ             # k-tiles covered by one aT[0] sub-tile

        # Availability model (us, relative to DMA boot) used to order the
        # block-0 wavefront: cumulative emitted bytes over ~0.358 MB/us plus
        # the serial DVE dequant pipeline.
        RATE = 0.358              # MB per us of HBM bandwidth
        DVE_G = 0.95              # us of DVE work per dequantized group

        avail_w = [0.0] * KT      # W_k ready time
        avail_a0 = [0.0] * AQ     # aT[0] quarter ready time
        avail_at = [0.0] * MBLK   # aT[mi] ready time (mi >= 1)

        with (
            tc.tile_pool(name="w", bufs=KT) as wpool,
            tc.tile_pool(name="zsb", bufs=3) as zsbpool,
            tc.tile_pool(name="qraw", bufs=3) as qpool,
            tc.tile_pool(name="deq", bufs=4) as dqpool,
            tc.tile_pool(name="a0", bufs=AQ) as a0pool,
            tc.tile_pool(name="at", bufs=10) as apool,
            tc.tile_pool(name="ot", bufs=4) as opool,
            tc.tile_pool(name="ps", bufs=MBLK, space="PSUM") as pspool,
        ):
            cum_mb = 0.0
            dve_free = 0.0
            at0 = [None] * AQ
            ats0 = [None] * MBLK
            w_tiles = []

            # PE warm-up: ~16 back-to-back matmuls on garbage data pull the
            # HAM clock gate to 8/8 (2.4 GHz) before real operands arrive,
            # so the ramp's real matmuls run at full rate. The dummy PSUM
            # slot is recycled by the wavefront (start=True resets it).
            warm_in = dqpool.tile([P, NL], f16, name="warm_in", tag="d")
            nc.gpsimd.memset(warm_in[:], 0.0)
            warm_ps = pspool.tile([P, NL], f32, name="warm_ps", tag="ps")
            for i in range(16):
                nc.tensor.matmul(
                    warm_ps[:],
                    warm_in[:, 0:P],
                    warm_in[:],
                    start=(i == 0),
                    stop=(i == 15),
                )

            def emit_at0_quarter(v):
                nonlocal cum_mb
                t = a0pool.tile([P, K // AQ], f16, name=f"at0q{v}", tag="a0")
                nc.sync.dma_start(t[:], aT[0][:, ts(v, K // AQ)])
                cum_mb += (P * K // AQ) * 2 / 1e6
                avail_a0[v] = cum_mb / RATE
                at0[v] = t

            def emit_at(mi):
                nonlocal cum_mb
                t = apool.tile([P, K], f16, name=f"at0_{mi}", tag="at")
                nc.sync.dma_start(t[:], aT[mi])
                cum_mb += (P * K) * 2 / 1e6
                avail_at[mi] = cum_mb / RATE
                ats0[mi] = t

            def emit_chunk(j, k_base, gpc):
                # Broadcasts issue from the Scalar sequencer and q loads
                # from GpSimd (SWDGE) so the ~0.6 us/DMA trigger cost isn't
                # serialized on the Sync sequencer with the aT loads.
                nonlocal cum_mb, dve_free
                zbc = zsbpool.tile([P, gpc * NL], f16, name=f"zbc{j}", tag="zb")
                nc.scalar.dma_start(
                    zbc[:],
                    zsm[:, k_base * NL : (k_base + gpc) * NL].partition_broadcast(P),
                )
                sbc = zsbpool.tile([P, gpc * NL], f16, name=f"sbc{j}", tag="sb")
                nc.scalar.dma_start(
                    sbc[:],
                    ssm[:, k_base * NL : (k_base + gpc) * NL].partition_broadcast(P),
                )
                cum_mb += 2 * (P * gpc * NL) * 2 / 1e6
                qt = qpool.tile([P, gpc, NL], i8, name=f"qt{j}", tag="qt")
                nc.gpsimd.dma_start(
                    qt[:],
                    q[k_base : k_base + gpc].rearrange("g p n -> p g n"),
                )
                cum_mb += (P * gpc * NL) / 1e6
                for g in range(gpc):
                    k = k_base + g
                    d = dqpool.tile([P, NL], f16, tag="d")
                    nc.vector.tensor_sub(
                        out=d[:], in0=qt[:, g, :], in1=zbc[:, ts(g, NL)]
                    )
                    wt = wpool.tile([P, NL], f16, tag="w")
                    nc.vector.tensor_mul(out=wt[:], in0=d[:], in1=sbc[:, ts(g, NL)])
                    w_tiles.append(wt)
                    dve_free = max(dve_free, cum_mb / RATE) + DVE_G
                    avail_w[k] = dve_free

            # Emission order: tiny chunk 0 leads (shortest path to W_0),
            # then aT[0]'s quarters, then the rest of the W pipeline with
            # block-0 aT tiles spread between chunks.
            k_base = 0
            emit_chunk(0, k_base, CHUNKS[0])
            k_base += CHUNKS[0]
            for v in range(AQ):
                emit_at0_quarter(v)
            for j in range(1, len(CHUNKS)):
                emit_chunk(j, k_base, CHUNKS[j])
                k_base += CHUNKS[j]
                if j < MBLK:
                    emit_at(j)

            def lhsT(mi, k):
                if mi == 0:
                    return at0[k // KQ][:, ts(k % KQ, P)]
                return ats0[mi][:, ts(k, P)]

            def avail_lhs(mi, k):
                return avail_a0[k // KQ] if mi == 0 else avail_at[mi]

            # Block 0: emit (mi, k) matmuls in estimated-availability order
            # so the PE stream stalls as little as possible during the ramp.
            pss = [
                pspool.tile([P, NL], f32, name=f"ps0_{i}", tag="ps")
                for i in range(MBLK)
            ]
            order = sorted(
                ((mi, k) for mi in range(MBLK) for k in range(KT)),
                key=lambda t: (
                    max(avail_lhs(t[0], t[1]), avail_w[t[1]]),
                    t[0],
                    t[1],
                ),
            )
            for mi, k in order:
                nc.tensor.matmul(
                    pss[mi][:],
                    lhsT(mi, k),
                    w_tiles[k][:],
                    start=(k == 0),
                    stop=(k == KT - 1),
                )
            for mi in range(MBLK):
                ot = opool.tile([P, NL], f32)
                nc.scalar.copy(ot[:], pss[mi][:])
                nc.scalar.dma_start(out[mi], ot[:])

            # Remaining m-tiles: m-outer, k-inner, inline epilogue.
            for m in range(MBLK, MT):
                at = apool.tile([P, K], f16, name=f"at_{m}", tag="at")
                nc.sync.dma_start(at[:], aT[m])
                ps = pspool.tile([P, NL], f32, name=f"ps_{m}", tag="ps")
                for k in range(KT):
                    nc.tensor.matmul(
                        ps[:],
                        at[:, ts(k, P)],
                        w_tiles[k][:],
                        start=(k == 0),
                        stop=(k == KT - 1),
                    )
                ot = opool.tile([P, NL], f32)
                nc.scalar.copy(ot[:], ps[:])
                nc.scalar.dma_start(out[m], ot[:])

    nc.compile()
    return nc


def _shard_inputs(a, q_weight, scales, zeros):
    """Host-side shard/layout. Pure slicing, transposition and replication."""
    # aT[m_out, k_in, k_out*128 + m_in] = a[m_out*128 + m_in, k_out*128 + k_in]
    aT = np.ascontiguousarray(
        a.reshape(MT, P, KT, P).transpose(0, 3, 2, 1)
    ).reshape(MT, P, K)
    # q values are 0..15: int8 container is lossless.
    q8 = q_weight.astype(np.int8)

    in_maps = []
    for c in range(NCORES):
        sl = slice(c * NL, (c + 1) * NL)
        q_c = np.ascontiguousarray(q8[:, sl]).reshape(KT, P, NL)
        z_c = np.ascontiguousarray(zeros[:, sl]).reshape(1, KT * NL)
        s_c = np.ascontiguousarray(scales[:, sl]).reshape(1, KT * NL)
        in_maps.append({"aT": aT, "q": q_c, "zsm": z_c, "ssm": s_c})
    return in_maps


def _run(inputs, trace=False):
    from concourse import bass_utils

    if "nc" not in _CACHE:
        _CACHE["nc"] = _build_nc()
    nc = _CACHE["nc"]

    a = np.asarray(inputs["a"], dtype=np.float16)
    q_weight = np.asarray(inputs["q_weight"], dtype=np.int32)
    scales = np.asarray(inputs["scales"], dtype=np.float16)
    zeros = np.asarray(inputs["zeros"], dtype=np.float16)

    in_maps = _shard_inputs(a, q_weight, scales, zeros)
    res = bass_utils.run_bass_kernel_spmd(
        nc, in_maps, core_ids=list(range(NCORES)), trace=trace
    )

    out = np.empty((M, N), dtype=np.float32)
    for c in range(NCORES):
        out[:, c * NL : (c + 1) * NL] = res.results[c]["out"].reshape(M, NL)
    return out, res


def kernel(**inputs) -> np.ndarray:
    out, _ = _run(inputs, trace=False)
    return out
